# revision 30
# baseline (speedup 1.0000x reference)
"""Distributed Trainium2 kernel for a contextual-loss module (raw Bass SPMD).

Math (per batch b, with y,x in [c=256, n=1024] layout, n = h*w):
    yn = y / ||y||_c ; xn = x / ||x||_c
    u  = yn^T @ xn                      (cosine similarity, [n, n])
    dist = 1 - u  (clip(0,2) never binds for randn inputs)
    dmin_j = max(1 - max_m u_jm, EPS)
    w = exp((1 - dist/dmin)/0.1) = exp(alpha_j * u'' + beta_j)   where
        u'' = y^T @ xn  (rows unnormalized),  ny_j = ||y_j||,
        m_j = min(smax_j - ny_j, -M_CLAMP)  (= -ny_j * dmin_j, clamped)
        am_j = 1/m_j,  alpha_j = -10*am_j,  beta_j = 10 + am_j*(10*ny_j)
    row max of w == 1 (exact whenever dmin > EPS), so
    cx_i_j = 1 / (sum_m w_jm + EPS)
    loss = mean_b(-log(mean_j cx_i_j + EPS))

Sharding: pure data parallel over batch, 8 batches per core on 8 cores.
Inputs are cast to bf16 on the host (they feed bf16 matmuls anyway),
halving DMA traffic and removing the on-device casts. Each core emits its
partial of sum(-log(...))/64; the host adds the 8 partials (equivalent to
the all-reduce of the scalar mean).

Engine split per batch (v1 baseline ran 267us; this version ~207us):
    sync  : DMA y,x (bf16, [128, 2KB] contiguous descriptors)
    gpsimd: x2=x^2, y2=y^2, y2s=y2c0+y2c1 (bf16), xn = x * nxinv
    tensor: ones-matmul partition reductions for ||x|| (replicated) and
            per-row-tile ||y||^2 columns, main y^T@xn matmuls (4 per tile:
            2 c-chunks x 2 moving halves; moving operand caps at 512),
            final cross-partition reduction of cx_i
    scalar: 1/sqrt via exp(-0.5*ln(.)) for x-norms, 10*||y|| via
            exp(0.5*ln+ln10), main exp with per-partition scale/bias and
            fused row-sum (accum_out into a flat [P, 64*8] buffer), single
            cx ln/exp pass at the end, final log
    vector: row-max over PSUM, then per pair the 4-stage temperature
            ladder (m -> reciprocal -> alpha/beta) interleaved across the
            two tiles so every producer/consumer has the required 1-op gap

The steady state is a software pipeline at ~2.2us/tile bound by the
latency loop te_main(g) -> DVE red+ladder -> ACT exp(g) -> te_main(g+3)
(u_ps is only 3 tiles deep: PSUM = 3x2 banks u + nx + small).

Raw Bass constraints honored (all verified on HW):
  - no 2-tensor DVE ops (GpSimd port contention corrupts them)
  - every DVE slice is 32B-aligned (stride-8 wide layout / col8 slices)
  - >=1 op between a DVE producer and DVE consumer (stale-read; violating
    this fails tolerance — re-verified this session)
  - walrus rejects instructions with multiple attached sync waits, so
    every wait is a standalone wait_ge (two-pass counting emitter)
  - reciprocal works only on contiguous APs (strided rank-3 gives garbage)
  - tensor_scalar divide / GP divide / custom-DVE ISA ops / DoubleRow-bf16
    / >512-wide matmul outputs are all rejected by this walrus build
"""

import numpy as np

N_CORES = 8
B_LOC = 8          # batches per core
C = 256
N = 1024
P = 128
NT = N // P        # 8 row tiles
NCH = C // P       # 2 contraction chunks
NP_ = NT // 2      # 4 tile pairs
EPS = 1e-5
M_CLAMP = 1.5e-4    # ~= EPS * ||y|| ; engages only on noise-broken rows

_cache = {}


class _Em:
    """Per-engine emitter: pass 1 counts sem values, pass 2 emits.

    Only DMA ops carry per-op increments (+16, HWDGE convention). For the
    compute engines an increment is attached only at mark() points — the
    only values anyone waits on — which keeps sem-inc traffic sparse.
    """

    def __init__(self, counting, engine, sems, cnt, marks, requested):
        self.counting = counting
        self.engine = engine
        self.sems = sems
        self.cnt = cnt
        self.marks = marks
        self.requested = requested
        self.last = None

    def wait(self, sem, label):
        if self.counting:
            self.requested.add(label)
            return
        if label not in self.marks:
            return  # b<0 dependency: nothing to wait on
        self.engine.wait_ge(self.sems[sem], self.marks[label])

    def do(self, sem, fn, by=1):
        if sem == "dma":
            self.cnt[sem] = self.cnt.get(sem, 0) + by
        if not self.counting:
            ins = fn(self.engine)
            if sem == "dma":
                ins.then_inc(self.sems[sem], by)
            self.last = ins

    def mark(self, label, sem):
        if sem == "dma":
            if self.counting:
                assert label not in self.marks, f"duplicate mark {label}"
                self.marks[label] = self.cnt.get(sem, 0)
            return
        self.cnt[sem] = self.cnt.get(sem, 0) + 1
        if self.counting:
            assert label not in self.marks, f"duplicate mark {label}"
            self.marks[label] = self.cnt[sem]
        else:
            assert self.last is not None
            self.last.then_inc(self.sems[sem], 1)
            self.last = None


def _build():
    from contextlib import ExitStack

    import concourse.bass as bass
    import concourse.mybir as mybir

    f32 = mybir.dt.float32
    bf16 = mybir.dt.bfloat16
    AX = mybir.AxisListType
    OP = mybir.AluOpType
    AF = mybir.ActivationFunctionType

    import os

    debug = os.environ.get("KDEBUG") == "1"

    nc = bass.Bass()

    y_ext = nc.dram_tensor("y_feat", [B_LOC, C, N], bf16, kind="ExternalInput")
    x_ext = nc.dram_tensor("x_feat", [B_LOC, C, N], bf16, kind="ExternalInput")
    out_ext = nc.dram_tensor("out", [1, 1], f32, kind="ExternalOutput")
    if debug:
        dbg_ext = {
            "dbg_cx": nc.dram_tensor("dbg_cx", [P, B_LOC * NT], f32,
                                     kind="ExternalOutput"),
            "dbg_sflat": nc.dram_tensor("dbg_sflat", [P, B_LOC * NT], f32,
                                        kind="ExternalOutput"),
            "dbg_smax": nc.dram_tensor("dbg_smax", [P, NT], f32,
                                       kind="ExternalOutput"),
            "dbg_ny": nc.dram_tensor("dbg_ny", [P, NT], f32,
                                     kind="ExternalOutput"),
            "dbg_alpha": nc.dram_tensor("dbg_alpha", [P, NT], f32,
                                        kind="ExternalOutput"),
            "dbg_beta": nc.dram_tensor("dbg_beta", [P, NT], f32,
                                       kind="ExternalOutput"),
            "dbg_nxinv": nc.dram_tensor("dbg_nxinv", [P, N], f32,
                                        kind="ExternalOutput"),
            "dbg_csum": nc.dram_tensor("dbg_csum", [1, B_LOC], f32,
                                       kind="ExternalOutput"),
        }

    with ExitStack() as ctx:
        sb = lambda nm, shape, dt: ctx.enter_context(nc.sbuf_tensor(nm, shape, dt))
        ps = lambda nm, shape, dt: ctx.enter_context(nc.psum_tensor(nm, shape, dt))
        sb2 = lambda nm, shape, dt: [sb(f"{nm}{i}", shape, dt) for i in range(2)]

        # per-batch tensors (slot = b % 2, y_b = b % 3); y/x arrive bf16
        y_b = [sb(f"y_b{i}", [P, NCH, N], bf16) for i in range(3)]
        x_b = sb2("x_b", [P, NCH, N], bf16)
        xn = sb2("xn_", [P, NCH, N], bf16)
        x2 = sb2("x2_", [P, NCH, N], bf16)
        y2 = sb2("y2_", [P, NCH, N], bf16)
        y2s = sb2("y2s", [P, N], bf16)
        nxinv = sb2("nxinv", [P, N], bf16)
        # Stride-8 "wide" layout for all per-row-tile scalars: tile t's
        # value lives at column 8*t, so every DVE slice is 32B-aligned.
        wide = lambda nm: sb2(nm, [P, NT * 8], f32)
        smax_w = wide("smaxw")
        e01_w = wide("e01w")
        alpha_w = wide("alphaw")
        beta_w = wide("betaw")
        ny_w = wide("nyw")
        negny_w = wide("negnyw")
        t_ln = sb("t_ln", [P, 512], f32)
        t_lny = sb("t_lny", [P, NT], f32)
        # flat row-sum accumulators for all 64 tiles (stride-8 wide)
        s_flat = sb("s_flat", [P, B_LOC * NT * 8], f32)
        t_cx = sb("t_cx", [P, B_LOC * NT], f32)
        cx_all = sb("cx_all", [P, B_LOC * NT], f32)
        w_scr = sb("w_scr", [P, N], bf16)
        junk = sb("junk", [P, 1], f32)

        col8 = lambda T, t: T[:, 8 * t:8 * t + 1]
        # [P, 2, 1] strided view of pair k (columns 16k and 16k+8)
        vpair = lambda T, k: T[:].rearrange("p (t e) -> p t e", e=8)[
            :, 2 * k:2 * k + 2, 0:1]
        vall = lambda T: T[:].rearrange("p (t e) -> p t e", e=8)[:, :, 0:1]
        vall64 = lambda T: T[:].rearrange("p (t e) -> p t e", e=8)[:, :, 0:1]
        ones_w = sb("ones_w", [P, P], bf16)
        ones_col = sb("ones_col", [P, 1], bf16)
        ones_f32 = sb("ones_f32", [P, 1], f32)
        eps_b = sb("eps_b", [P, 1], f32)
        ln10_b = sb("ln10_b", [P, 1], f32)
        csum = sb("csum", [1, B_LOC], f32)
        lnb = sb("lnb", [1, B_LOC], f32)
        lsum = sb("lsum", [1, 1], f32)
        partial = sb("partial", [1, 1], f32)

        # PSUM: 3x u (2 banks each) + nx (1 bank) + small (1 bank) = 8 banks
        u_ps = [ps(f"u_ps{i}", [P, N], f32) for i in range(3)]
        nx_ps = ps("nx_ps", [P, 512], f32)
        small_ps = ps("small_ps", [P, 64], f32)

        sems = {
            "dma": ctx.enter_context(nc.semaphore("dma_sem")),
            "gp": ctx.enter_context(nc.semaphore("gp_sem")),
            "te": ctx.enter_context(nc.semaphore("te_sem")),
            "act": ctx.enter_context(nc.semaphore("act_sem")),
            "dve": ctx.enter_context(nc.semaphore("dve_sem")),
        }

        # Bass(target_bir_lowering=False) skips the init-time semaphore
        # clear, so sems carry values from previous NEFF executions and
        # every wait_ge threshold would be wrong. Clear them explicitly,
        # then an NRT-level barrier keeps the other engines from racing
        # ahead of the clear.
        from concourse.bass import compact_to_ranges

        for sem_range in compact_to_ranges(
            [s for s in nc._kernel_sem_range if s not in nc.barrier_sems]
        ):
            nc.gpsimd.dma_reset(sem_range)
            nc.gpsimd.sem_clear(sem_range)
        nc._nrt_pseudo_barrier()

        # ---------------- engine programs ----------------

        def prog_sync(E):
            for b in range(B_LOC):
                s = b % 2
                sy = b % 3
                # y_b slot WAR: TE mains of b-3 read it as stationary
                E.wait("te", f"te_main_{(b - 3) * NT + NT - 1}")
                for c in range(NCH):
                    E.do("dma", lambda e, sy=sy, b=b, c=c: e.dma_start(
                        y_b[sy][:, c, :], y_ext[b, c * P:(c + 1) * P, :]), by=16)
                    E.mark(f"dma_y{c}_{b}", "dma")
                E.wait("gp", f"gp_xn_{b - 2}")
                for c in range(NCH):
                    E.do("dma", lambda e, s=s, b=b, c=c: e.dma_start(
                        x_b[s][:, c, :], x_ext[b, c * P:(c + 1) * P, :]), by=16)
                    E.mark(f"dma_x{c}_{b}", "dma")
            E.wait("dve", "dve_final")
            E.do("dma", lambda e: e.dma_start(out_ext[:, :], partial[:]), by=16)
            if debug:
                s1 = (B_LOC - 1) % 2
                items = [("dbg_cx", cx_all[:]),
                         ("dbg_sflat", vall64(s_flat)),
                         ("dbg_smax", vall(smax_w[s1])),
                         ("dbg_ny", vall(ny_w[s1])),
                         ("dbg_alpha", vall(alpha_w[s1])),
                         ("dbg_beta", vall(beta_w[s1])),
                         ("dbg_nxinv", nxinv[s1][:]),
                         ("dbg_csum", csum[:])]
                for nm, src in items:
                    def dbg_dma(e, nm=nm, src=src):
                        with nc.allow_non_contiguous_dma(reason="debug dump"):
                            return e.dma_start(dbg_ext[nm][:], src)
                    E.do("dma", dbg_dma, by=16)

        def prog_gpsimd(E):
            # GP is the tightest-budget engine: only x2, y2 and xn live here
            # (y2s moved to TE as accumulating ny matmuls). Order per step:
            # xn(b) (deadline: mains(b)), then next batch's x2/y2 prefetch.
            E.do("gp", lambda e: e.memset(ones_w[:], 1.0))
            E.do("gp", lambda e: e.memset(ones_col[:], 1.0))
            E.do("gp", lambda e: e.memset(ones_f32[:], 1.0))
            E.do("gp", lambda e: e.memset(eps_b[:], EPS))
            E.do("gp", lambda e: e.memset(ln10_b[:], float(np.log(10.0))))

            def x2_gp(E, b):
                s = b % 2
                for c in range(NCH):
                    E.wait("dma", f"dma_x{c}_{b}")
                E.do("gp", lambda e, s=s: e.tensor_mul(
                    x2[s][:], x_b[s][:], x_b[s][:]))
                E.mark(f"gp_x2_{b}", "gp")

            def y2_gp(E, b):
                s = b % 2
                sy = b % 3
                for c in range(NCH):
                    E.wait("dma", f"dma_y{c}_{b}")
                E.do("gp", lambda e, s=s, sy=sy: e.tensor_mul(
                    y2[s][:], y_b[sy][:], y_b[sy][:]))
                E.do("gp", lambda e, s=s: e.tensor_add(
                    y2s[s][:], y2[s][:, 0, :], y2[s][:, 1, :]))
                E.mark(f"gp_y2s_{b}", "gp")

            x2_gp(E, 0)
            y2_gp(E, 0)
            for b in range(B_LOC):
                s = b % 2
                # xn in m-halves: the h0 ops start as soon as ACT's h0
                # exp lands, so the full-xn mark fires ~2us earlier (mainly
                # shortens the batch-0 startup chain)
                E.wait("act", f"act_nxh0_{b}")
                for c in range(NCH):
                    E.do("gp", lambda e, s=s, c=c: e.tensor_tensor(
                        xn[s][:, c, 0:512], x_b[s][:, c, 0:512],
                        nxinv[s][:, 0:512], OP.mult))
                E.wait("act", f"act_nxinv_{b}")
                for c in range(NCH):
                    E.do("gp", lambda e, s=s, c=c: e.tensor_tensor(
                        xn[s][:, c, 512:1024], x_b[s][:, c, 512:1024],
                        nxinv[s][:, 512:1024], OP.mult))
                E.mark(f"gp_xn_{b}", "gp")
                if b + 1 < B_LOC:
                    x2_gp(E, b + 1)
                    y2_gp(E, b + 1)

        def prog_tensor(E):
            def nxh0_te(E, b):
                s = b % 2
                E.wait("gp", f"gp_x2_{b}")
                # nx_ps WAR: ACT's ln of the previous batch's h1 must be done
                E.wait("act", f"act_lnh1_{b - 1}")
                for c in range(NCH):
                    E.do("te" if c == NCH - 1 else None,
                         lambda e, s=s, c=c: e.matmul(
                             nx_ps[:], ones_w[:],
                             x2[s][:, c, 0:512],
                             start=(c == 0), stop=(c == NCH - 1)))
                E.mark(f"te_nxh0_{b}", "te")

            def nxh1_te(E, b):
                s = b % 2
                E.wait("act", f"act_lnh0_{b}")
                for c in range(NCH):
                    E.do("te" if c == NCH - 1 else None,
                         lambda e, s=s, c=c: e.matmul(
                             nx_ps[:], ones_w[:],
                             x2[s][:, c, 512:1024],
                             start=(c == 0), stop=(c == NCH - 1)))
                E.mark(f"te_nxh1_{b}", "te")

            def ny_te(E, b):
                # ||y||^2 columns [128, NT] in small_ps[:, 0:NT]
                s = b % 2
                E.wait("gp", f"gp_y2s_{b}")
                E.wait("act", f"act_lnny_{b - 1}")
                for t in range(NT):
                    E.do("te" if t == NT - 1 else None,
                         lambda e, s=s, t=t: e.matmul(
                             small_ps[:, t:t + 1],
                             y2s[s][:, t * P:(t + 1) * P],
                             ones_col[:],
                             start=True, stop=True))
                E.mark(f"te_ny_{b}", "te")

            nxh0_te(E, 0)
            nxh1_te(E, 0)
            ny_te(E, 0)
            for b in range(B_LOC):
                s = b % 2
                sy = b % 3
                # main tiles; batch b+1's norm matmuls are hoisted into the
                # middle so the x-norm chain (gp x2 -> te nx -> act ln/exp ->
                # gp xn) closes before mains(b+1) need xn
                E.wait("gp", f"gp_xn_{b}")
                for t in range(NT):
                    g = b * NT + t
                    E.wait("act", f"act_exp_{g - 3}")
                    for c in range(NCH):
                        for h in range(2):
                            E.do("te" if (c == NCH - 1 and h == 1) else None,
                                 lambda e, s=s, sy=sy, t=t, c=c, h=h, g=g: e.matmul(
                                     u_ps[g % 3][:, h * 512:(h + 1) * 512],
                                     y_b[sy][:, c, t * P:(t + 1) * P],
                                     xn[s][:, c, h * 512:(h + 1) * 512],
                                     start=(c == 0), stop=(c == NCH - 1)))
                    E.mark(f"te_main_{g}", "te")
                    if b + 1 < B_LOC:
                        if t == 3:
                            nxh0_te(E, b + 1)
                        elif t == 4:
                            nxh1_te(E, b + 1)
                        elif t == 5:
                            ny_te(E, b + 1)
            # final partition-reduction of cx_i
            E.wait("act", "act_cx")
            E.do("te", lambda e: e.matmul(
                small_ps[:1, :], ones_f32[:], cx_all[:], start=True, stop=True))
            E.mark("te_loss", "te")

        def prog_scalar(E):
            def nx_act_h0(E, b):
                # x-norm ln/exp chain, h0 half: split from h1 so the two
                # [P,512] op pairs land in different ACT idle windows and
                # the pair exps (which gate TE via u_ps) shift less
                s = b % 2
                # WAR on nxinv slot vs gp xn readers of b-2
                E.wait("gp", f"gp_xn_{b - 2}")
                E.wait("te", f"te_nxh0_{b}")
                E.do("act", lambda e: e.activation(t_ln[:], nx_ps[:], AF.Ln))
                E.mark(f"act_lnh0_{b}", "act")
                E.do("act", lambda e, s=s: e.activation(
                    nxinv[s][:, 0:512], t_ln[:], AF.Exp, scale=-0.5))
                E.mark(f"act_nxh0_{b}", "act")

            def nx_act_h1(E, b):
                s = b % 2
                E.wait("te", f"te_nxh1_{b}")
                E.do("act", lambda e: e.activation(t_ln[:], nx_ps[:], AF.Ln))
                E.mark(f"act_lnh1_{b}", "act")
                E.do("act", lambda e, s=s: e.activation(
                    nxinv[s][:, 512:1024], t_ln[:], AF.Exp, scale=-0.5))
                E.mark(f"act_nxinv_{b}", "act")

            def ny_act(E, b):
                s = b % 2
                # WAR on ny_w slot vs dve negny reader of b-2
                E.wait("dve", f"dve_negny_{b - 2}")
                E.wait("te", f"te_ny_{b}")
                E.do("act", lambda e: e.activation(
                    t_lny[:], small_ps[:, 0:NT], AF.Ln))
                E.mark(f"act_lnny_{b}", "act")
                # tenny = 10*||y|| = exp(0.5*ln(Ny^2) + ln 10), strided
                E.do("act", lambda e, s=s: e.activation(
                    vall(ny_w[s]), t_lny[:].rearrange(
                        "p (t e) -> p t e", e=1), AF.Exp, scale=0.5,
                    bias=ln10_b[:]))
                E.mark(f"act_ny_{b}", "act")

            nx_act_h0(E, 0)
            nx_act_h1(E, 0)
            ny_act(E, 0)
            for b in range(B_LOC):
                s = b % 2
                for k in range(NP_):
                    for t in (2 * k, 2 * k + 1):
                        g = b * NT + t
                        # per-tile gate: exp(2k) only needs beta0 of pair k
                        E.wait("dve", f"dve_b{t % 2}_{b}_{k}")
                        E.do("act", lambda e, s=s, t=t, g=g: e.activation(
                            w_scr[:], u_ps[g % 3][:], AF.Exp,
                            bias=col8(beta_w[s], t),
                            scale=col8(alpha_w[s], t),
                            accum_out=col8(s_flat, g)))
                        E.mark(f"act_exp_{g}", "act")
                    if b + 1 < B_LOC:
                        # hoisted early: closes the x-norm chain in time for
                        # gp xn(b+1) to finish before mains(b+1)
                        if k == 0:
                            nx_act_h0(E, b + 1)
                        elif k == 1:
                            nx_act_h1(E, b + 1)
                        elif k == 2:
                            ny_act(E, b + 1)

            # cx_i = 1/(S+EPS) via exp(-ln(S+EPS)) for all 64 tiles at once.
            # Two spacers first: the last exp's accum_out commits after the
            # main output stream; a short-distance ACT read sees stale data.
            E.do("act", lambda e: e.activation(junk[:], junk[:], AF.Identity))
            E.do("act", lambda e: e.activation(junk[:], junk[:], AF.Identity))
            E.do("act", lambda e: e.activation(
                t_cx[:].rearrange("p (t e) -> p t e", e=1),
                vall64(s_flat), AF.Ln, bias=eps_b[:]))
            E.do("act", lambda e: e.activation(
                cx_all[:], t_cx[:], AF.Exp, scale=-1.0))
            # spacer so the TE loss-matmul's operand fetch doesn't race
            # the tail of the cx_all write
            E.do("act", lambda e: e.activation(junk[:], junk[:], AF.Identity))
            E.mark("act_cx", "act")
            # final log
            E.wait("dve", "dve_csum")
            E.do("act", lambda e: e.activation(
                lnb[:], csum[:], AF.Ln, scale=1.0 / N, bias=eps_b[:1, :]))
            E.mark("act_lnb", "act")

        def prog_vector(E):
            # DVE constraints baked into this schedule (all verified on HW):
            #  - no 2-tensor DVE ops (GpSimd port contention corrupts them)
            #  - every DVE slice is 32B-aligned (stride-8 wide layout)
            #  - >=1 op between a DVE producer and DVE consumer (stale-read)
            #  - reciprocal only on contiguous APs ([P,1] col8 slices)
            def J(E):
                E.do("dve", lambda e: e.tensor_scalar_mul(junk[:], junk[:], 1.0))

            for b in range(B_LOC):
                s = b % 2
                # negny = -||y|| for this batch's chain ops
                E.wait("act", f"act_ny_{b}")
                E.do("dve", lambda e, s=s: e.tensor_scalar_mul(
                    vall(negny_w[s]), vall(ny_w[s]), -0.1))
                E.mark(f"dve_negny_{b}", "dve")
                for k in range(NP_):
                    # J-free ladder: every op's producer is exactly 2 ops
                    # back, so the >=1-op stale-read gap holds throughout:
                    # red0, red1, e0, e1, cl0, cl1, rec0, rec1, b0, b1
                    g0 = b * NT + 2 * k
                    E.wait("te", f"te_main_{g0}")
                    E.do("dve", lambda e, s=s, k=k, g0=g0: e.tensor_reduce(
                        col8(smax_w[s], 2 * k), u_ps[g0 % 3][:],
                        axis=AX.X, op=OP.max))
                    E.wait("te", f"te_main_{g0 + 1}")
                    E.do("dve", lambda e, s=s, k=k, g0=g0: e.tensor_reduce(
                        col8(smax_w[s], 2 * k + 1), u_ps[(g0 + 1) % 3][:],
                        axis=AX.X, op=OP.max))
                    # m_t = min(smax_t - ny_t, -M_CLAMP)  (merged clamp)
                    for t in (2 * k, 2 * k + 1):
                        E.do("dve", lambda e, s=s, t=t: e.tensor_scalar(
                            col8(e01_w[s], t), col8(smax_w[s], t),
                            col8(negny_w[s], t), -M_CLAMP,
                            op0=OP.add, op1=OP.min))
                    # am = 1/m (negative), stored over the spent smax col
                    for t in (2 * k, 2 * k + 1):
                        E.do("dve", lambda e, s=s, t=t: e.reciprocal(
                            out=col8(smax_w[s], t), in_=col8(e01_w[s], t)))
                    # alpha = -10*am, beta = am*(10*ny) + 10; the beta mark
                    # gates ACT's exp for its tile (alpha lands just before)
                    for t in (2 * k, 2 * k + 1):
                        E.do("dve", lambda e, s=s, t=t: e.tensor_scalar_mul(
                            col8(alpha_w[s], t), col8(smax_w[s], t), -10.0))
                        E.do("dve", lambda e, s=s, t=t: e.tensor_scalar(
                            col8(beta_w[s], t), col8(smax_w[s], t),
                            col8(ny_w[s], t), 10.0,
                            op0=OP.mult, op1=OP.add))
                        E.mark(f"dve_b{t % 2}_{b}_{k}", "dve")
            # final
            E.wait("te", "te_loss")
            E.do("dve", lambda e: e.tensor_reduce(
                csum[:], small_ps[:1, :].rearrange("p (b t) -> p b t", t=NT),
                axis=AX.X, op=OP.add))
            J(E)
            E.mark("dve_csum", "dve")
            E.wait("act", "act_lnb")
            E.do("dve", lambda e: e.tensor_reduce(
                lsum[:], lnb[:], axis=AX.X, op=OP.add))
            J(E)
            E.do("dve", lambda e: e.tensor_scalar_mul(
                partial[:], lsum[:], -1.0 / (B_LOC * N_CORES)))
            J(E)
            E.mark("dve_final", "dve")

        # ---------------- two passes ----------------
        progs = {
            "sync": prog_sync,
            "gpsimd": prog_gpsimd,
            "tensor": prog_tensor,
            "scalar": prog_scalar,
            "vector": prog_vector,
        }
        marks = {}
        requested = set()
        for name, prog in progs.items():
            prog(_Em(True, None, sems, {}, marks, requested))
        for lbl in requested:
            if lbl not in marks:
                assert "-" in lbl, f"waited label {lbl} never marked"

        with nc.Block() as block:
            @block.sync
            def _(eng):
                prog_sync(_Em(False, eng, sems, {}, marks, requested))

            @block.gpsimd
            def _(eng):
                prog_gpsimd(_Em(False, eng, sems, {}, marks, requested))

            @block.tensor
            def _(eng):
                prog_tensor(_Em(False, eng, sems, {}, marks, requested))

            @block.scalar
            def _(eng):
                prog_scalar(_Em(False, eng, sems, {}, marks, requested))

            @block.vector
            def _(eng):
                prog_vector(_Em(False, eng, sems, {}, marks, requested))

    return nc


def _ensure_ntff_hook():
    """This image's antenv package lacks axon_hooks; bass_utils imports it
    unconditionally when BASS_TRACE is set. Recreate it from the boot
    module's ctypes implementation so tracing works."""
    import sys
    import types

    if "antenv.axon_hooks" not in sys.modules:
        mod = types.ModuleType("antenv.axon_hooks")
        box = [None]

        def set_axon_ntff_profile_hook(h):
            box[0] = h

        def get_axon_ntff_profile_hook():
            if box[0] is None:
                try:
                    from trn_agent_boot.trn_boot import _ntff_profile_via_ctypes

                    box[0] = _ntff_profile_via_ctypes("/opt/axon/libaxon_pjrt.so")
                except Exception:
                    return None
            return box[0]

        mod.set_axon_ntff_profile_hook = set_axon_ntff_profile_hook
        mod.get_axon_ntff_profile_hook = get_axon_ntff_profile_hook
        sys.modules["antenv.axon_hooks"] = mod
        try:
            import antenv

            antenv.axon_hooks = mod
        except Exception:
            pass
    import concourse.bass_utils as bu

    bu.upload_artifacts = lambda tmpdir: str(tmpdir)  # zero-egress container


def kernel(y_feat: np.ndarray, x_feat: np.ndarray) -> np.ndarray:
    _ensure_ntff_hook()
    from concourse.bass_utils import run_bass_kernel_spmd

    if "nc" not in _cache:
        _cache["nc"] = _build()
    nc = _cache["nc"]

    import ml_dtypes

    bf = ml_dtypes.bfloat16
    y = np.ascontiguousarray(
        np.asarray(y_feat, np.float32).reshape(64, C, N).astype(bf))
    x = np.ascontiguousarray(
        np.asarray(x_feat, np.float32).reshape(64, C, N).astype(bf))
    in_maps = [
        {"y_feat": y[i * B_LOC:(i + 1) * B_LOC], "x_feat": x[i * B_LOC:(i + 1) * B_LOC]}
        for i in range(N_CORES)
    ]
    res = run_bass_kernel_spmd(nc, in_maps, core_ids=list(range(N_CORES)))
    _cache["last_results"] = res
    total = np.float32(0.0)
    for r in res.results:
        total += np.float32(r["out"].reshape(-1)[0])
    return np.float32(total).reshape(())
